# revision 9
# baseline (speedup 1.0000x reference)
"""Trainium2 kernel for nn_BlockSparseMatrix: block-sparse -> dense reconstruction
plus CSR/CSC index building.

Strategy (8 NeuronCores, SPMD):
  - Shard the 256x256 block grid by block-row: core m owns block-rows
    [32m, 32m+32) = dense rows [1024m, 1024m+1024), a 32 MiB output stripe.
    Since nnz positions are sorted row-major, each core's blocks are a
    contiguous slice of `data`.
  - The stripe is built in 8 tiles of (128 partitions x 8192 f32) = 4 block
    rows each; each tile splits into 8 "groups" of 128 grid slots
    (4 block-rows g x 32 block-cols c'). Per group:
      1. zero-fill a staging tile (128, 1024),
      2. one indirect-DMA gather: partition 32g+c' receives the whole 4 KiB
         block for slot (g, c') from DRAM (one full-rate descriptor per
         occupied slot; empty slots carry an OOB index and are skipped,
         leaving zeros) -- the per-partition dynamic source address is the
         mechanism that makes placement data-dependent,
      3. one DVE stream-transpose (32x32 blocks) swaps the within-32
         partition coordinate c' with the within-32 free coordinate q:
         partition becomes 32g+q (the dense row), free becomes 32b+c',
      4. one strided copy permutes free (b,c') -> (c',b), landing the
         transposed block at its column slot in the output tile.
  - One contiguous 4 MiB HWDGE DMA per tile writes the dense stripe.
  - The tiny index outputs (CSR/CSC pointers, a few hundred KB total) are
    computed on host; the 256 MiB dense tensor dominates all memory traffic
    and is produced on device.

HBM traffic per core: 8 MiB block reads + 32 MiB dense writes.
"""

import sys
from contextlib import ExitStack

import numpy as np

for _p in (
    "/root/.axon_site",
    "/root/.axon_site/_ro/trn_rl_repo",
    "/root/.axon_site/_ro/pypackages",
):
    if _p not in sys.path:
        sys.path.append(_p)

BH = BW = 32
X = Y = 256
M = 8                 # cores
ROWS_PER_CORE = X // M          # 32 block-rows
TILES = ROWS_PER_CORE // 4      # 8 tiles of 4 block-rows
WINDOWS = Y // 32               # 8 col-windows of 32 block-cols per tile
GROUPS = TILES * WINDOWS        # 64 gather groups per core
F = Y * BW                      # 8192 f32 free dim per output tile
K_MAX = 2560                    # padded blocks per core stripe (marker = K_MAX)

_compiled_nc = None

# test harness hooks: extra kwargs for run_bass_kernel_spmd and the last results
run_kwargs: dict = {}
last_results = None


def _build_body(tc, dense_ap, blk_ap, idx_ap, ctx: ExitStack):
    import concourse.bass as bass
    import concourse.mybir as mybir

    nc = tc.nc
    f32 = mybir.dt.float32
    idxp = ctx.enter_context(tc.tile_pool(name="idxp", bufs=1))
    zp = ctx.enter_context(tc.tile_pool(name="zp", bufs=1))
    sp = ctx.enter_context(tc.tile_pool(name="sp", bufs=8))
    rp = ctx.enter_context(tc.tile_pool(name="rp", bufs=4))
    tp = ctx.enter_context(tc.tile_pool(name="tp", bufs=3))

    idx_sb = idxp.tile([128, GROUPS], mybir.dt.int32)
    nc.sync.dma_start(idx_sb[:], idx_ap[:])

    zero = zp.tile([128, 1024], f32)
    nc.vector.memset(zero[:], 0.0)

    for t in range(TILES):
        T = tp.tile([128, F], f32, tag="T")
        for w in range(WINDOWS):
            gi = t * WINDOWS + w
            stg = sp.tile([128, 1024], f32, tag="stg")
            # zero-fill mostly on ACT; a quarter on Pool to keep ACT under DVE
            if gi % 4 == 3:
                nc.gpsimd.memset(stg[:], 0.0)
            else:
                nc.scalar.copy(stg[:], zero[:])
            # dynamic block placement: partition 32g+c' <- block at
            # (block-row 4t+g, col 32w+c'), one 4 KiB descriptor per slot
            nc.gpsimd.indirect_dma_start(
                out=stg[:],
                out_offset=None,
                in_=blk_ap[:],
                in_offset=bass.IndirectOffsetOnAxis(
                    ap=idx_sb[:, gi:gi + 1], axis=0
                ),
                bounds_check=K_MAX - 1,
                oob_is_err=False,
            )
            # stream-transpose with permuted out AP writes the transposed
            # block of slot (g, c') directly at column slot c' of T.  The
            # strided write costs ~1.5x on DVE, so a third of the groups
            # instead do a contiguous transpose + strided ACT copy.
            dst = T[:, 1024 * w:1024 * (w + 1)].rearrange("p (c b) -> p b c", b=32)
            if gi % 3 == 0:
                R = rp.tile([128, 1024], f32, tag="R")
                nc.vector.transpose(R[:], stg[:])
                nc.scalar.copy(dst, R[:].rearrange("p (b c) -> p b c", b=32))
            else:
                nc.vector.transpose(dst, stg[:])
        nc.sync.dma_start(dense_ap[128 * t:128 * (t + 1), :], T[:])


def _get_program():
    global _compiled_nc
    if _compiled_nc is not None:
        return _compiled_nc

    import concourse.bacc as bacc
    import concourse.mybir as mybir
    import concourse.tile as tile

    nc = bacc.Bacc("TRN2", target_bir_lowering=False, debug=False, num_devices=M)
    blk = nc.dram_tensor(
        "blk", [K_MAX, BH * BW], mybir.dt.float32, kind="ExternalInput"
    ).ap()
    idx = nc.dram_tensor(
        "idx", [128, GROUPS], mybir.dt.int32, kind="ExternalInput"
    ).ap()
    dense = nc.dram_tensor(
        "dense", [ROWS_PER_CORE * BH, F], mybir.dt.float32, kind="ExternalOutput"
    ).ap()

    with tile.TileContext(nc) as tc, ExitStack() as ctx:
        _build_body(tc, dense, blk, idx, ctx)
    nc.compile()
    _compiled_nc = nc
    return nc


def _host_indices(mask: np.ndarray, n: int):
    """Everything except the dense tensor, mirroring reference() on host."""
    i32 = np.int32
    rows, cols = np.nonzero(mask)  # row-major order
    block_ptr = np.arange(n)

    blocks = np.stack([cols, rows], axis=1).reshape(-1).astype(i32)

    row_counts = np.zeros(X + 1, np.int64)
    np.add.at(row_counts, rows + 1, 1)
    row_start_ends_a = np.cumsum(row_counts).astype(i32)
    cols_a = np.stack([cols, block_ptr], axis=1).astype(i32)

    bi = np.zeros(X * Y, np.int64)
    bi[rows * Y + cols] = block_ptr + 1
    bit = bi.reshape(X, Y).T.reshape(-1)
    tpos = np.nonzero(bit)[0]
    block_ptr_t = (bit[tpos] - 1).astype(i32)

    rows_t, cols_t = np.nonzero(mask.T)
    col_counts = np.zeros(Y + 1, np.int64)
    np.add.at(col_counts, rows_t + 1, 1)
    col_start_ends_b = np.cumsum(col_counts).astype(i32)
    rows_b = np.stack([cols_t, block_ptr_t], axis=1).astype(i32)

    return rows, cols, blocks, cols_a, row_start_ends_a, rows_b, col_start_ends_b


def _shard_inputs(rows, cols, data):
    """Per-core (blk, idx) arrays."""
    in_maps = []
    stripe_bounds = np.searchsorted(rows, np.arange(M + 1) * ROWS_PER_CORE)
    for m in range(M):
        s, e = int(stripe_bounds[m]), int(stripe_bounds[m + 1])
        k = e - s
        assert k <= K_MAX, f"stripe {m} has {k} blocks > K_MAX={K_MAX}"
        blk = np.zeros((K_MAX, BH * BW), np.float32)
        blk[:k] = data[s * BH:e * BH].reshape(k, BH * BW)

        grid = np.full((ROWS_PER_CORE, Y), K_MAX, np.int64)
        grid[rows[s:e] - m * ROWS_PER_CORE, cols[s:e]] = np.arange(k)
        # idx[32g + c', 8t + w] = grid[4t + g, 32w + c'] (or K_MAX marker)
        g4 = grid.reshape(TILES, 4, WINDOWS, 32)        # [t, g, w, c']
        idx = (
            g4.transpose(1, 3, 0, 2).reshape(128, GROUPS).astype(np.int32)
        )
        in_maps.append({"blk": blk, "idx": idx})
    return in_maps


def kernel(block_mask, data):
    global last_results
    mask = np.asarray(block_mask, dtype=bool)
    data = np.asarray(data, dtype=np.float32)
    n = data.shape[0] // BH

    (
        rows,
        cols,
        blocks,
        cols_a,
        row_start_ends_a,
        rows_b,
        col_start_ends_b,
    ) = _host_indices(mask, n)

    in_maps = _shard_inputs(rows, cols, data)
    nc = _get_program()

    from concourse.bass_utils import run_bass_kernel_spmd

    res = run_bass_kernel_spmd(nc, in_maps, core_ids=list(range(M)), **run_kwargs)
    last_results = res
    dense = np.concatenate([r["dense"] for r in res.results], axis=0)

    return dense, blocks, cols_a, row_start_ends_a, rows_b, col_start_ends_b


# revision 11
# speedup vs baseline: 1.3696x; 1.3696x over previous
"""Trainium2 kernel for nn_BlockSparseMatrix: block-sparse -> dense reconstruction
plus CSR/CSC index building.

Strategy (8 NeuronCores, SPMD):
  - Shard the 256x256 block grid by block-row: core m owns block-rows
    [32m, 32m+32) = dense rows [1024m, 1024m+1024), a 32 MiB output stripe.
    Since nnz positions are sorted row-major, each core's blocks are a
    contiguous slice of `data`.
  - The stripe is built in 8 tiles of (128 partitions x 8192 f32) = 4 block
    rows each; each tile splits into 8 "groups" of 128 grid slots
    (4 block-rows g x 32 block-cols c'). Per group:
      1. zero-fill a staging tile (128, 1024),
      2. one indirect-DMA gather: partition 32g+c' receives the whole 4 KiB
         block for slot (g, c') from DRAM (one full-rate descriptor per
         occupied slot; empty slots carry an OOB index and are skipped,
         leaving zeros) -- the per-partition dynamic source address is the
         mechanism that makes placement data-dependent,
      3. one DVE stream-transpose (32x32 blocks) swaps the within-32
         partition coordinate c' with the within-32 free coordinate q:
         partition becomes 32g+q (the dense row), free becomes 32b+c',
      4. one strided copy permutes free (b,c') -> (c',b), landing the
         transposed block at its column slot in the output tile.
  - One contiguous 4 MiB HWDGE DMA per tile writes the dense stripe.
  - The tiny index outputs (CSR/CSC pointers, a few hundred KB total) are
    computed on host; the 256 MiB dense tensor dominates all memory traffic
    and is produced on device.

HBM traffic per core: 8 MiB block reads + 32 MiB dense writes.
"""

import sys
from contextlib import ExitStack

import numpy as np

for _p in (
    "/root/.axon_site",
    "/root/.axon_site/_ro/trn_rl_repo",
    "/root/.axon_site/_ro/pypackages",
):
    if _p not in sys.path:
        sys.path.append(_p)

BH = BW = 32
X = Y = 256
M = 8                 # cores
ROWS_PER_CORE = X // M          # 32 block-rows
TILES = ROWS_PER_CORE // 4      # 8 tiles of 4 block-rows
WINDOWS = Y // 32               # 8 col-windows of 32 block-cols per tile
GROUPS = TILES * WINDOWS        # 64 gather groups per core
F = Y * BW                      # 8192 f32 free dim per output tile
K_MAX = 2560                    # padded blocks per core stripe (marker = K_MAX)

_compiled_nc = None

# test harness hooks: extra kwargs for run_bass_kernel_spmd and the last results
run_kwargs: dict = {}
last_results = None


def _build_body(tc, dense_ap, blk_ap, idx_ap, ctx: ExitStack):
    import concourse.bass as bass
    import concourse.mybir as mybir

    nc = tc.nc
    f32 = mybir.dt.float32
    idxp = ctx.enter_context(tc.tile_pool(name="idxp", bufs=1))
    zp = ctx.enter_context(tc.tile_pool(name="zp", bufs=1))
    sp = ctx.enter_context(tc.tile_pool(name="sp", bufs=12))
    tp = ctx.enter_context(tc.tile_pool(name="tp", bufs=12))

    idx_sb = idxp.tile([128, GROUPS], mybir.dt.int32)
    nc.sync.dma_start(idx_sb[:], idx_ap[:])

    zero = zp.tile([128, 1024], f32)
    nc.vector.memset(zero[:], 0.0)

    for t in range(TILES):
        for w in range(WINDOWS):
            gi = t * WINDOWS + w
            stg = sp.tile([128, 1024], f32, tag="stg")
            # zero-fill on ACT (otherwise idle); DVE/Pool are the busy engines
            nc.scalar.copy(stg[:], zero[:])
            # dynamic block placement: partition 32g+c' <- block at
            # (block-row 4t+g, col 32w+c'), one 4 KiB descriptor per slot
            nc.gpsimd.indirect_dma_start(
                out=stg[:],
                out_offset=None,
                in_=blk_ap[:],
                in_offset=bass.IndirectOffsetOnAxis(
                    ap=idx_sb[:, gi:gi + 1], axis=0
                ),
                bounds_check=K_MAX - 1,
                oob_is_err=False,
            )
            # stream-transpose with permuted out AP: writes the transposed
            # block of slot (g, c') directly at its column slot in W
            W = tp.tile([128, 1024], f32, tag="W")
            nc.vector.transpose(
                W[:].rearrange("p (c b) -> p b c", b=32), stg[:]
            )
            nc.sync.dma_start(
                dense_ap[128 * t:128 * (t + 1), 1024 * w:1024 * (w + 1)], W[:]
            )


def _get_program():
    global _compiled_nc
    if _compiled_nc is not None:
        return _compiled_nc

    import concourse.bacc as bacc
    import concourse.mybir as mybir
    import concourse.tile as tile

    nc = bacc.Bacc("TRN2", target_bir_lowering=False, debug=False, num_devices=M)
    blk = nc.dram_tensor(
        "blk", [K_MAX, BH * BW], mybir.dt.float32, kind="ExternalInput"
    ).ap()
    idx = nc.dram_tensor(
        "idx", [128, GROUPS], mybir.dt.int32, kind="ExternalInput"
    ).ap()
    dense = nc.dram_tensor(
        "dense", [ROWS_PER_CORE * BH, F], mybir.dt.float32, kind="ExternalOutput"
    ).ap()

    with tile.TileContext(nc) as tc, ExitStack() as ctx:
        _build_body(tc, dense, blk, idx, ctx)
    nc.compile()
    _compiled_nc = nc
    return nc


def _host_indices(mask: np.ndarray, n: int):
    """Everything except the dense tensor, mirroring reference() on host."""
    i32 = np.int32
    rows, cols = np.nonzero(mask)  # row-major order
    block_ptr = np.arange(n)

    blocks = np.stack([cols, rows], axis=1).reshape(-1).astype(i32)

    row_counts = np.zeros(X + 1, np.int64)
    np.add.at(row_counts, rows + 1, 1)
    row_start_ends_a = np.cumsum(row_counts).astype(i32)
    cols_a = np.stack([cols, block_ptr], axis=1).astype(i32)

    bi = np.zeros(X * Y, np.int64)
    bi[rows * Y + cols] = block_ptr + 1
    bit = bi.reshape(X, Y).T.reshape(-1)
    tpos = np.nonzero(bit)[0]
    block_ptr_t = (bit[tpos] - 1).astype(i32)

    rows_t, cols_t = np.nonzero(mask.T)
    col_counts = np.zeros(Y + 1, np.int64)
    np.add.at(col_counts, rows_t + 1, 1)
    col_start_ends_b = np.cumsum(col_counts).astype(i32)
    rows_b = np.stack([cols_t, block_ptr_t], axis=1).astype(i32)

    return rows, cols, blocks, cols_a, row_start_ends_a, rows_b, col_start_ends_b


def _shard_inputs(rows, cols, data):
    """Per-core (blk, idx) arrays."""
    in_maps = []
    stripe_bounds = np.searchsorted(rows, np.arange(M + 1) * ROWS_PER_CORE)
    for m in range(M):
        s, e = int(stripe_bounds[m]), int(stripe_bounds[m + 1])
        k = e - s
        assert k <= K_MAX, f"stripe {m} has {k} blocks > K_MAX={K_MAX}"
        blk = np.zeros((K_MAX, BH * BW), np.float32)
        blk[:k] = data[s * BH:e * BH].reshape(k, BH * BW)

        grid = np.full((ROWS_PER_CORE, Y), K_MAX, np.int64)
        grid[rows[s:e] - m * ROWS_PER_CORE, cols[s:e]] = np.arange(k)
        # idx[32g + c', 8t + w] = grid[4t + g, 32w + c'] (or K_MAX marker)
        g4 = grid.reshape(TILES, 4, WINDOWS, 32)        # [t, g, w, c']
        idx = (
            g4.transpose(1, 3, 0, 2).reshape(128, GROUPS).astype(np.int32)
        )
        in_maps.append({"blk": blk, "idx": idx})
    return in_maps


def kernel(block_mask, data):
    global last_results
    mask = np.asarray(block_mask, dtype=bool)
    data = np.asarray(data, dtype=np.float32)
    n = data.shape[0] // BH

    (
        rows,
        cols,
        blocks,
        cols_a,
        row_start_ends_a,
        rows_b,
        col_start_ends_b,
    ) = _host_indices(mask, n)

    in_maps = _shard_inputs(rows, cols, data)
    nc = _get_program()

    from concourse.bass_utils import run_bass_kernel_spmd

    res = run_bass_kernel_spmd(nc, in_maps, core_ids=list(range(M)), **run_kwargs)
    last_results = res
    dense = np.concatenate([r["dense"] for r in res.results], axis=0)

    return dense, blocks, cols_a, row_start_ends_a, rows_b, col_start_ends_b


# revision 13
# speedup vs baseline: 1.4952x; 1.0917x over previous
"""Trainium2 kernel for nn_BlockSparseMatrix: block-sparse -> dense reconstruction
plus CSR/CSC index building.

Strategy (8 NeuronCores, SPMD):
  - Shard the 256x256 block grid by block-row: core m owns block-rows
    [32m, 32m+32) = dense rows [1024m, 1024m+1024), a 32 MiB output stripe.
    Since nnz positions are sorted row-major, each core's blocks are a
    contiguous slice of `data`.
  - The stripe is built in 8 tiles of (128 partitions x 8192 f32) = 4 block
    rows each; each tile splits into 8 "groups" of 128 grid slots
    (4 block-rows g x 32 block-cols c'). Per group:
      1. zero-fill a staging tile (128, 1024),
      2. one indirect-DMA gather: partition 32g+c' receives the whole 4 KiB
         block for slot (g, c') from DRAM (one full-rate descriptor per
         occupied slot; empty slots carry an OOB index and are skipped,
         leaving zeros) -- the per-partition dynamic source address is the
         mechanism that makes placement data-dependent,
      3. one DVE stream-transpose (32x32 blocks) swaps the within-32
         partition coordinate c' with the within-32 free coordinate q:
         partition becomes 32g+q (the dense row), free becomes 32b+c',
      4. one strided copy permutes free (b,c') -> (c',b), landing the
         transposed block at its column slot in the output tile.
  - One contiguous 4 MiB HWDGE DMA per tile writes the dense stripe.
  - The tiny index outputs (CSR/CSC pointers, a few hundred KB total) are
    computed on host; the 256 MiB dense tensor dominates all memory traffic
    and is produced on device.

HBM traffic per core: 8 MiB block reads + 32 MiB dense writes.
"""

import sys
from contextlib import ExitStack

import numpy as np

for _p in (
    "/root/.axon_site",
    "/root/.axon_site/_ro/trn_rl_repo",
    "/root/.axon_site/_ro/pypackages",
):
    if _p not in sys.path:
        sys.path.append(_p)

BH = BW = 32
X = Y = 256
M = 8                 # cores
ROWS_PER_CORE = X // M          # 32 block-rows
TILES = ROWS_PER_CORE // 4      # 8 tiles of 4 block-rows
WINDOWS = Y // 32               # 8 col-windows of 32 block-cols per tile
GROUPS = TILES * WINDOWS        # 64 gather groups per core
F = Y * BW                      # 8192 f32 free dim per output tile
K_MAX = 2560                    # padded blocks per core stripe (marker = K_MAX)

_compiled_nc = None

# test harness hooks: extra kwargs for run_bass_kernel_spmd and the last results
run_kwargs: dict = {}
last_results = None


def _build_body(tc, dense_ap, blk_ap, idx_ap, ctx: ExitStack):
    import concourse.bass as bass
    import concourse.mybir as mybir

    nc = tc.nc
    f32 = mybir.dt.float32
    idxp = ctx.enter_context(tc.tile_pool(name="idxp", bufs=1))
    zp = ctx.enter_context(tc.tile_pool(name="zp", bufs=1))
    sp = ctx.enter_context(tc.tile_pool(name="sp", bufs=16))
    rp = ctx.enter_context(tc.tile_pool(name="rp", bufs=6))
    tp = ctx.enter_context(tc.tile_pool(name="tp", bufs=16))

    idx_sb = idxp.tile([128, GROUPS], mybir.dt.int32)
    nc.sync.dma_start(idx_sb[:], idx_ap[:])

    zero = zp.tile([128, 1024], f32)
    nc.vector.memset(zero[:], 0.0)

    for t in range(TILES):
        for w in range(WINDOWS):
            gi = t * WINDOWS + w
            stg = sp.tile([128, 1024], f32, tag="stg")
            # zero-fill on ACT (otherwise idle); DVE/Pool are the busy engines
            nc.scalar.copy(stg[:], zero[:])
            # dynamic block placement: partition 32g+c' <- block at
            # (block-row 4t+g, col 32w+c'), one 4 KiB descriptor per slot
            nc.gpsimd.indirect_dma_start(
                out=stg[:],
                out_offset=None,
                in_=blk_ap[:],
                in_offset=bass.IndirectOffsetOnAxis(
                    ap=idx_sb[:, gi:gi + 1], axis=0
                ),
                bounds_check=K_MAX - 1,
                oob_is_err=False,
            )
            # stream-transpose with permuted out AP: writes the transposed
            # block of slot (g, c') directly at its column slot in W.  The
            # strided write costs ~1.6x on DVE, so every 4th window does a
            # contiguous transpose + strided ACT copy instead.
            W = tp.tile([128, 1024], f32, tag="W")
            if gi % 4 == 3:
                R = rp.tile([128, 1024], f32, tag="R")
                nc.vector.transpose(R[:], stg[:])
                nc.scalar.copy(
                    W[:].rearrange("p (c b) -> p b c", b=32),
                    R[:].rearrange("p (b c) -> p b c", c=32),
                )
            else:
                nc.vector.transpose(
                    W[:].rearrange("p (c b) -> p b c", b=32), stg[:]
                )
            nc.sync.dma_start(
                dense_ap[128 * t:128 * (t + 1), 1024 * w:1024 * (w + 1)], W[:]
            )


def _get_program():
    global _compiled_nc
    if _compiled_nc is not None:
        return _compiled_nc

    import concourse.bacc as bacc
    import concourse.mybir as mybir
    import concourse.tile as tile

    nc = bacc.Bacc("TRN2", target_bir_lowering=False, debug=False, num_devices=M)
    blk = nc.dram_tensor(
        "blk", [K_MAX, BH * BW], mybir.dt.float32, kind="ExternalInput"
    ).ap()
    idx = nc.dram_tensor(
        "idx", [128, GROUPS], mybir.dt.int32, kind="ExternalInput"
    ).ap()
    dense = nc.dram_tensor(
        "dense", [ROWS_PER_CORE * BH, F], mybir.dt.float32, kind="ExternalOutput"
    ).ap()

    with tile.TileContext(nc) as tc, ExitStack() as ctx:
        _build_body(tc, dense, blk, idx, ctx)
    nc.compile()
    _compiled_nc = nc
    return nc


def _host_indices(mask: np.ndarray, n: int):
    """Everything except the dense tensor, mirroring reference() on host."""
    i32 = np.int32
    rows, cols = np.nonzero(mask)  # row-major order
    block_ptr = np.arange(n)

    blocks = np.stack([cols, rows], axis=1).reshape(-1).astype(i32)

    row_counts = np.zeros(X + 1, np.int64)
    np.add.at(row_counts, rows + 1, 1)
    row_start_ends_a = np.cumsum(row_counts).astype(i32)
    cols_a = np.stack([cols, block_ptr], axis=1).astype(i32)

    bi = np.zeros(X * Y, np.int64)
    bi[rows * Y + cols] = block_ptr + 1
    bit = bi.reshape(X, Y).T.reshape(-1)
    tpos = np.nonzero(bit)[0]
    block_ptr_t = (bit[tpos] - 1).astype(i32)

    rows_t, cols_t = np.nonzero(mask.T)
    col_counts = np.zeros(Y + 1, np.int64)
    np.add.at(col_counts, rows_t + 1, 1)
    col_start_ends_b = np.cumsum(col_counts).astype(i32)
    rows_b = np.stack([cols_t, block_ptr_t], axis=1).astype(i32)

    return rows, cols, blocks, cols_a, row_start_ends_a, rows_b, col_start_ends_b


def _shard_inputs(rows, cols, data):
    """Per-core (blk, idx) arrays."""
    in_maps = []
    stripe_bounds = np.searchsorted(rows, np.arange(M + 1) * ROWS_PER_CORE)
    for m in range(M):
        s, e = int(stripe_bounds[m]), int(stripe_bounds[m + 1])
        k = e - s
        assert k <= K_MAX, f"stripe {m} has {k} blocks > K_MAX={K_MAX}"
        blk = np.zeros((K_MAX, BH * BW), np.float32)
        blk[:k] = data[s * BH:e * BH].reshape(k, BH * BW)

        grid = np.full((ROWS_PER_CORE, Y), K_MAX, np.int64)
        grid[rows[s:e] - m * ROWS_PER_CORE, cols[s:e]] = np.arange(k)
        # idx[32g + c', 8t + w] = grid[4t + g, 32w + c'] (or K_MAX marker)
        g4 = grid.reshape(TILES, 4, WINDOWS, 32)        # [t, g, w, c']
        idx = (
            g4.transpose(1, 3, 0, 2).reshape(128, GROUPS).astype(np.int32)
        )
        in_maps.append({"blk": blk, "idx": idx})
    return in_maps


def kernel(block_mask, data):
    global last_results
    mask = np.asarray(block_mask, dtype=bool)
    data = np.asarray(data, dtype=np.float32)
    n = data.shape[0] // BH

    (
        rows,
        cols,
        blocks,
        cols_a,
        row_start_ends_a,
        rows_b,
        col_start_ends_b,
    ) = _host_indices(mask, n)

    in_maps = _shard_inputs(rows, cols, data)
    nc = _get_program()

    from concourse.bass_utils import run_bass_kernel_spmd

    res = run_bass_kernel_spmd(nc, in_maps, core_ids=list(range(M)), **run_kwargs)
    last_results = res
    dense = np.concatenate([r["dense"] for r in res.results], axis=0)

    return dense, blocks, cols_a, row_start_ends_a, rows_b, col_start_ends_b
